# revision 2
# baseline (speedup 1.0000x reference)
"""Trainium2 Bass kernel for nn_Cross_modal_ContrastiveLoss6.

Math: the reference loss only depends on per-class means of the two
modalities (every entry of the N x N distance matrix is determined by the
class pair), so the whole computation reduces to:

  1. raw per-class segment sums R[c,d], T[c,d]  (memory-bound)
  2. the three 128x128 class Gram matrices P1 = R R^T, P2 = T T^T, P3 = R T^T
  3. tiny 128x128 class-pair loss math with the class counts

Device strategy (8 cores, feature/d-sharded so no cross-core collective is
needed): core k takes columns [256k, 256k+256) of both modal tensors and
computes the full-N segment sums for its d-chunk with one-hot matmuls on
the PE.  The data ships as fp8 e4m3 (quantization alone gives ~7e-4 final
rel err, well under the 2e-2 gate), split into two DMA streams so both
HW-DGE queues (Sync + Scalar) stream concurrently at aggregate ~400 GB/s:

  stream A (sync queue):   sample blocks  0..15, 4 chunks x 256 KiB
  stream B (scalar queue): targets meta, then blocks 16..31, 4 chunks

One-hot stationaries are generated on-device off the critical path: a
16 KiB targets DMA rides first on the scalar queue, gpsimd builds the
fp32 iota row, and the DVE emits one-hots 4 blocks per tensor_tensor
(is_equal against a stride-0-broadcast target AP) in exactly the order
the PE consumes chunks.  The PE runs DoubleRow fp8 matmuls gated only on
(x chunk landed, one-hot ready), accumulating [128 classes, 512] =
(R|T sums) in one PSUM bank.  A short junk-matmul warm-up at block entry
lifts the PE HAM clock-gate while the first DMA chunks are in flight.
The PSUM->SBUF bf16 cast is split between the DVE (low half) and the
Scalar/ACT engine (high half) so the two output half-DMAs issue in
parallel on both queues.  The host forms the three Grams and does the
count scaling + sqrt/relu/weighted mean (<0.1% of the FLOPs) in float64.
"""

import contextlib

import numpy as np
import ml_dtypes

import concourse.bass as bass
import concourse.mybir as mybir
from concourse.bass_utils import run_bass_kernel_spmd

N = 4096
D = 2048
C = 128
MARGIN = 0.5
NCORES = 8
DCHUNK = D // NCORES          # 256 feature columns per core
P = 128                       # partitions / sample-block size
NB = N // P                   # 32 sample blocks
BLK_BYTES = 2 * DCHUNK        # 512 fp8 bytes per partition per block (x1|x2)
BPC = 4                       # blocks per DMA chunk (2 KiB/partition = 256 KiB)
NCHUNK_Q = 4                  # chunks per queue (2 queues x 4 chunks x 4 blocks)
HALF = NB // 2                # 16 blocks per stream
NWARM = 4                     # junk matmuls to lift the PE HAM clock gate

F32 = mybir.dt.float32
BF16 = mybir.dt.bfloat16
F8 = mybir.dt.float8e4
NPF8 = ml_dtypes.float8_e4m3  # IEEE e4m3 (bias 7, +-240 max) == TRN float8e4
DR = mybir.MatmulPerfMode.DoubleRow

_PROGRAM = None

# Chunk consumption order: alternate stream A / stream B so the PE tracks
# both queues as they land.  Entry = (stream, chunk_idx); stream A chunk j
# covers blocks [4j, 4j+4), stream B chunk j covers blocks [16+4j, 16+4j+4).
CONSUME = [("a", 0), ("b", 0), ("a", 1), ("b", 1),
           ("a", 2), ("b", 2), ("a", 3), ("b", 3)]


def _chunk_blocks(stream: str, j: int) -> int:
    return (0 if stream == "a" else HALF) + BPC * j


def _build_program() -> bass.Bass:
    """Raw-bass program: fp8 x on two HW-DGE queues, on-device one-hots,
    16 DoubleRow matmuls, split bf16 cast, two parallel output half-DMAs.
    """
    nc = bass.Bass()

    # meta[p, b] = targets[b*128 + p] as fp32 (tiny, lands first)
    meta_in = nc.declare_dram_parameter("meta", [P, NB], F32, isOutput=False)
    # x streams: [p, (blk, j)] : j<256 -> modal1[blk*128+p, dchunk[j]] fp8,
    #                            j>=256 -> modal2[...]
    xa_in = nc.declare_dram_parameter("xa", [P, HALF * BLK_BYTES], F8, isOutput=False)
    xb_in = nc.declare_dram_parameter("xb", [P, HALF * BLK_BYTES], F8, isOutput=False)
    # sums[:, 0:256] = R segment sums, [:, 256:512] = T (bf16)
    sums_out = nc.declare_dram_parameter("sums", [P, 512], BF16, isOutput=True)

    with contextlib.ExitStack() as stack:
        x_t = stack.enter_context(nc.sbuf_tensor([P, NB, BLK_BYTES], F8))
        oh_t = stack.enter_context(nc.sbuf_tensor([P, NB, C], F8))
        iota_t = stack.enter_context(nc.sbuf_tensor([P, BPC, C], F32))
        meta_t = stack.enter_context(nc.sbuf_tensor([P, NB], F32))
        warm_t = stack.enter_context(nc.sbuf_tensor([P, 640], F8))
        out_t = stack.enter_context(nc.sbuf_tensor([P, 512], BF16))
        psum_acc = stack.enter_context(nc.psum_tensor([P, 512], F32))
        psum_warm = stack.enter_context(nc.psum_tensor([P, 512], F32))

        def sem(name):
            return stack.enter_context(nc.semaphore(name))

        meta_sem = sem("meta_dma")
        xa_sems = [sem(f"xa_dma_{j}") for j in range(NCHUNK_Q)]
        xb_sems = [sem(f"xb_dma_{j}") for j in range(NCHUNK_Q)]
        iota_sem = sem("iota_done")
        oh_gen = sem("oh_gen")
        pe_done = sem("pe_done")
        cast_lo = sem("cast_lo")
        out_sems = [sem("out_lo"), sem("out_hi")]

        # Raw-bass semaphores are NOT cleared by the framework preamble;
        # stale values from a previous run of this same program would
        # satisfy our waits early.  Clear them, then fence with the NRT
        # pseudo barrier so no engine reaches a wait before the clears.
        all_sems = ([meta_sem] + xa_sems + xb_sems
                    + [iota_sem, oh_gen, pe_done, cast_lo] + out_sems)
        nums = sorted(h.num for h in all_sems)
        assert nums == list(range(nums[0], nums[0] + len(nums))), nums
        sem_range = range(nums[0], nums[-1] + 1)
        nc.gpsimd.dma_reset(sem_range)
        nc.gpsimd.sem_clear(sem_range)
        nc._nrt_pseudo_barrier()

        with nc.Block(no_gpsimd_drain=True) as block:

            @block.gpsimd
            def _(gpsimd: bass.BassEngine):
                # iota[p, r, c] = c (fp32, exact for 0..127), shared by all
                # 4-block one-hot ops on the DVE.
                nc.gpsimd.iota(
                    iota_t[:, :, :],
                    pattern=[[0, BPC], [1, C]],
                    base=0,
                    channel_multiplier=0,
                    allow_small_or_imprecise_dtypes=True,
                )
                gpsimd.drain().then_inc(iota_sem, 1)

            @block.sync
            def _(sync: bass.BassEngine):
                for j in range(NCHUNK_Q):
                    b0 = _chunk_blocks("a", j)
                    fl = slice(b0 * BLK_BYTES, (b0 + BPC) * BLK_BYTES)
                    sync.dma_start(
                        out=x_t[:, b0 : b0 + BPC, :], in_=xa_in[:, fl]
                    ).then_inc(xa_sems[j], 16)
                sync.wait_ge(cast_lo, 1)
                sync.dma_start(
                    out=sums_out[:, 0:256], in_=out_t[:, 0:256]
                ).then_inc(out_sems[0], 16)
                sync.wait_ge(out_sems[0], 16)

            @block.scalar
            def _(scalar: bass.BassEngine):
                scalar.dma_start(out=meta_t[:], in_=meta_in[:]).then_inc(
                    meta_sem, 16
                )
                for j in range(NCHUNK_Q):
                    b0 = _chunk_blocks("b", j) - HALF
                    fl = slice(b0 * BLK_BYTES, (b0 + BPC) * BLK_BYTES)
                    b0 += HALF
                    scalar.dma_start(
                        out=x_t[:, b0 : b0 + BPC, :], in_=xb_in[:, fl]
                    ).then_inc(xb_sems[j], 16)
                # high-half cast on the ACT engine, then its own output DMA
                scalar.wait_ge(pe_done, 1)
                nc.scalar.copy(out_t[:, 256:512], psum_acc[:, 256:512])
                scalar.drain()
                scalar.dma_start(
                    out=sums_out[:, 256:512], in_=out_t[:, 256:512]
                ).then_inc(out_sems[1], 16)
                scalar.wait_ge(out_sems[1], 16)

            @block.tensor
            def _(tensor: bass.BassEngine):
                # Lift the PE HAM clock gate on junk data while the first
                # DMA chunks are in flight.
                for _ in range(NWARM):
                    nc.tensor.matmul(
                        psum_warm[:],
                        warm_t[:, 0:128],
                        warm_t[:, 128:640],
                        start=True,
                        stop=True,
                    )
                for k, (stream, j) in enumerate(CONSUME):
                    tensor.wait_ge(oh_gen, k + 1)
                    tensor.wait_ge(xa_sems[j] if stream == "a" else xb_sems[j], 16)
                    b0 = _chunk_blocks(stream, j)
                    for pr in (b0, b0 + 2):
                        nc.tensor.matmul(
                            psum_acc[:],
                            oh_t[:, pr : pr + 2, :],
                            x_t[:, pr : pr + 2, :],
                            start=(k == 0 and pr == b0),
                            stop=(k == len(CONSUME) - 1 and pr == b0 + 2),
                            perf_mode=DR,
                        )
                tensor.drain().then_inc(pe_done, 1)

            @block.vector
            def _(vector: bass.BassEngine):
                # oh[p, b, c] = (targets[b*128+p] == c) as fp8 (0/1 exact),
                # 4 blocks per op, in PE consumption order.
                vector.wait_ge(iota_sem, 1)
                vector.wait_ge(meta_sem, 16)
                for stream, j in CONSUME:
                    b0 = _chunk_blocks(stream, j)
                    tgt_b = (
                        meta_t[:, b0 : b0 + BPC]
                        .unsqueeze(2)
                        .broadcast_to([P, BPC, C])
                    )
                    nc.vector.tensor_tensor(
                        oh_t[:, b0 : b0 + BPC, :],
                        iota_t[:, :, :],
                        tgt_b,
                        mybir.AluOpType.is_equal,
                    ).then_inc(oh_gen, 1)
                # low-half cast; sync picks it up for the output DMA
                vector.wait_ge(pe_done, 1)
                nc.vector.tensor_copy(out_t[:, 0:256], psum_acc[:, 0:256])
                vector.drain().then_inc(cast_lo, 1)

    return nc


def _get_program() -> bass.Bass:
    global _PROGRAM
    if _PROGRAM is None:
        _PROGRAM = _build_program()
    return _PROGRAM


def _make_in_maps(modal1, modal2, targets):
    x1 = np.asarray(modal1, dtype=np.float32).astype(NPF8)
    x2 = np.asarray(modal2, dtype=np.float32).astype(NPF8)
    targets = np.asarray(targets)

    meta = np.ascontiguousarray(targets.reshape(NB, P).T.astype(np.float32))

    in_maps = []
    for k in range(NCORES):
        sl = slice(k * DCHUNK, (k + 1) * DCHUNK)
        # [128, NB, 512] : [p, b, 0:256] = x1 chunk, [p, b, 256:512] = x2 chunk
        a = x1[:, sl].reshape(NB, P, DCHUNK).transpose(1, 0, 2)
        b = x2[:, sl].reshape(NB, P, DCHUNK).transpose(1, 0, 2)
        x = np.concatenate([a, b], axis=2)
        xa = np.ascontiguousarray(x[:, :HALF].reshape(P, HALF * BLK_BYTES))
        xb = np.ascontiguousarray(x[:, HALF:].reshape(P, HALF * BLK_BYTES))
        in_maps.append({"meta": meta, "xa": xa, "xb": xb})
    return in_maps


def _finish_on_host(sums_list, targets):
    """Recombine per-core sums, form class Grams, and do the class-pair loss."""
    P1 = np.zeros((C, C), np.float64)
    P2 = np.zeros((C, C), np.float64)
    P3 = np.zeros((C, C), np.float64)
    for s in sums_list:
        s = np.asarray(s, np.float64)
        R = s[:, 0:256]                      # [class, d-chunk]
        T = s[:, 256:512]
        P1 += R @ R.T
        P2 += T @ T.T
        P3 += R @ T.T

    n = np.bincount(targets, minlength=C).astype(np.float64)
    u = 1.0 / np.maximum(n, 1.0)

    S_CC = P1 + P2 + P3 + P3.T  # (R+T)(R+T)^T
    uu = np.outer(u, u)
    A1 = 0.5 * uu * (P1 + P3)    # meanR . ctr
    A2 = 0.5 * uu * (P2 + P3.T)  # meanT . ctr
    nR = u * u * np.diag(P1)
    nT = u * u * np.diag(P2)
    nCtr = 0.25 * u * u * np.diag(S_CC)

    W = np.outer(n, n)
    eye = np.eye(C)
    total = 0.0
    for A, nrm in ((A1, nR), (A2, nT)):
        sq = np.maximum(nrm[:, None] + nCtr[None, :] - 2.0 * A, 1e-12)
        d = np.sqrt(sq)
        dd = np.sqrt(d + 1e-10)
        term = eye * sq + (1.0 - eye) * np.maximum(MARGIN - dd, 0.0) ** 2
        total += (W * term).sum() / (float(N) * float(N))
    return np.asarray(total, dtype=np.float32)


def kernel(modal1_inputs, modal2_inputs, targets):
    nc = _get_program()
    in_maps = _make_in_maps(modal1_inputs, modal2_inputs, targets)
    res = run_bass_kernel_spmd(nc, in_maps, list(range(NCORES)))
    sums_list = [
        np.asarray(res.results[k]["sums"], dtype=np.float32) for k in range(NCORES)
    ]
    return _finish_on_host(sums_list, np.asarray(targets))
